# revision 30
# baseline (speedup 1.0000x reference)
"""TRN2 Bass kernel for multi-head self-attention with RoPE (causal).

Problem: B=4, S=2048, D=768, H=12 heads of dk=64, fp32 in/out.

Sharding: 8 cores = 4 batches x 2 head-groups of 6 heads. Each core computes
QKV projections for its 6 heads, RoPE, causal attention, and a partial
output projection; the host sums the two partials per batch.

v4: TRANSPOSED-SCORES rewrite. The v3 kernel spent ~525us of Sync/Scalar
dispatch + 16 DMA queues on XBAR-transposing P [q,k] -> [k,q] for the AV
matmul, starving the PE (51% busy, p-state never ramped). v4 computes the
precise scores directly transposed, S^T[k,q] = K Q^T, so exp(S^T) IS the
AV rhs and no transpose of score-sized data ever happens:

  - cheap-max pass [q,k]: one 2-term matmul per 512-block
    ((qhi+qlo)@khi via band layout), DVE row-max (mask fused into the
    last block via tensor_tensor_reduce with a right-aligned [0|tri]
    mask), per-128-q-tile maxes -> negate -> bf16 -> one [128,128] XBAR
    transpose per (head, q-group) -> row [1,512] of -mhat.
    Empirically |rowmax_cheap - rowmax_precise| <= 7 raw-score units;
    the exp window tolerates ~60, and any consistent bias cancels in the
    normalization, so the cheap max only has to bound, not match.
  - precise S^T per k-tile j (same 3 terms as v3 = qhi@khi + qlo@khi +
    qhi@klo): mm1 = k_hl[khi|klo bands] x q_hh[qhi|qhi], mm2 =
    k_ho[khi;ones][65] x ql_b[qlo;-mhat][65] -- the 65th contract row
    adds -mhat to every score for free, solving the "per-column bias"
    problem of the transposed layout.
  - mask_t on diagonal tiles, ACT exp (scale=1/8, no bias) -> pts bf16,
    AV accumulates v_ext[128,65-with-ones] x pts into [65,512] psum;
    row 64 = softmax denominator for free; reciprocal+broadcast
    normalize into av_all.
  - PE p-state care: TRN2's tensor clock ramps 0.65->1.2->2.4GHz with
    ~3us of continuous busy; every idle gap halves throughput for the
    next 3us. A weave scheduler interleaves (proj bursts) x (cheap of
    slot t+1) x (S^T/AV of slot t) at matmul granularity so the PE
    stream stays dense and all DVE/ACT post-processing hides under it.
"""

import sys

sys.path.insert(0, "/opt/trn_rl_repo")

from collections import deque
from contextlib import ExitStack

import ml_dtypes
import numpy as np

import concourse.bass as bass
import concourse.tile as tile
from concourse import bacc, mybir
from concourse.bass_utils import run_bass_kernel_spmd

F32 = mybir.dt.float32
BF16 = mybir.dt.bfloat16
bf16 = ml_dtypes.bfloat16

B, D, H, DK = 4, 768, 12, 64
NHC = 6          # heads per core
NPAIR = 3        # head pairs per core
DSUB = 6         # d_in subtiles of 128
CPC = NHC * DK   # 384 head-dims per core

SUB = mybir.AluOpType.subtract
ADD = mybir.AluOpType.add
MAX = mybir.AluOpType.max
EXP = mybir.ActivationFunctionType.Exp


def _build(S=2048, trace_label=""):
    NQT = S // 128       # 16 q-tiles
    NG = NQT // 4        # 4 q-groups per head (512 q cols each)
    nc = bacc.Bacc("TRN2", target_bir_lowering=False, debug=False,
                   num_devices=8)

    def din(name, shape, dt):
        return nc.dram_tensor(name, shape, dt, kind="ExternalInput").ap()

    xh_d = din("xh", [128, DSUB, S], BF16)
    xl_d = din("xl", [128, DSUB, S], BF16)
    wqh_d = din("wqh", [128, DSUB, CPC], BF16)
    wql_d = din("wql", [128, DSUB, CPC], BF16)
    wkh_d = din("wkh", [128, DSUB, CPC], BF16)
    wkl_d = din("wkl", [128, DSUB, CPC], BF16)
    wv_d = din("wvT", [128, DSUB, CPC], BF16)
    wo_d = din("woT", [128, NPAIR, D], BF16)
    cos_d = din("cos_t", [128, S], F32)
    sin_d = din("sin_t", [128, S], F32)
    m512_d = din("mask512", [128, 512], F32)   # [zeros(384) | triu128]
    maskt_d = din("mask_t", [128, 128], F32)   # tril(-1e9, -1): [k,q] diag
    out_d = nc.dram_tensor("out", [S, D], F32, kind="ExternalOutput").ap()

    with tile.TileContext(nc) as tc, ExitStack() as ctx:
        # ---------- persistent SBUF ----------
        pers = ctx.enter_context(tc.tile_pool(name="pers", bufs=1))

        def load(pool, dr, name):
            t = pool.tile(list(dr.shape), dr.dtype, tag=f"L{name}")
            nc.sync.dma_start(t[:], dr[:])
            return t

        mask512 = load(pers, m512_d, "m512")
        mask_t = load(pers, maskt_d, "maskt")
        cos_t = load(pers, cos_d, "cos")
        sin_t = load(pers, sin_d, "sin")

        # v with an appended ones column per head: AV matmul row 64 = sum(p)
        v_ext = pers.tile([128, NQT, NHC, 65], BF16, tag="v_ext")
        av_all = pers.tile([128, NPAIR, S], BF16, tag="av_all")

        # pools that span the V phase and the main loop (prefetch of the
        # first Q/K weights + x chunk overlaps the V projection)
        wsl = ctx.enter_context(tc.tile_pool(name="wsl", bufs=2))
        bx = ctx.enter_context(tc.tile_pool(name="bx", bufs=2))

        def load_wslices(p):
            ws = {}
            for nm, dr in (("qh", wqh_d), ("ql", wql_d),
                           ("kh", wkh_d), ("kl", wkl_d)):
                t = wsl.tile([128, DSUB, 128], BF16, tag=f"w{nm}", name=nm)
                nc.sync.dma_start(t[:], dr[:, :, bass.ts(p, 128)])
                ws[nm] = t
            return ws

        def load_xc(c_seq):
            # prefetched one chunk ahead of its proj consumer
            xc = bx.tile([128, 2, DSUB, 512], BF16, tag="xc", name="xc")
            ssl = bass.ts(c_seq % (S // 512), 512)
            nc.sync.dma_start(xc[:, 0], xh_d[:, :, ssl])
            nc.sync.dma_start(xc[:, 1], xl_d[:, :, ssl])
            return xc

        ws_first = load_wslices(0)
        xc_first = load_xc(0)

        # ---------- V projection upfront ----------
        with tc.tile_pool(name="vw", bufs=1) as vw, \
             tc.tile_pool(name="vx", bufs=2) as vx, \
             tc.tile_pool(name="pv", bufs=2, space="PSUM") as pvs:
            wv = load(vw, wv_d, "wv")
            nc.gpsimd.memset(v_ext[:, :, :, 64:65], 1.0)

            def load_xv(sc_i):
                xv = vx.tile([128, DSUB, 512], BF16, tag="xv")
                nc.sync.dma_start(xv[:], xh_d[:, :, bass.ts(sc_i, 512)])
                return xv

            xv_cur = load_xv(0)
            for sc_i in range(S // 512):
                xv = xv_cur
                if sc_i < S // 512 - 1:
                    xv_cur = load_xv(sc_i + 1)
                for st4 in range(4):
                    st = sc_i * 4 + st4
                    psv = pvs.tile([128, 512], F32, tag="pv")
                    for t in range(DSUB):
                        nc.tensor.matmul(
                            psv[:, 0:CPC],
                            xv[:, t, bass.ts(st4, 128)], wv[:, t, :],
                            start=(t == 0), stop=(t == DSUB - 1),
                        )
                    nc.scalar.copy(out=v_ext[:, st, :, 0:64], in_=psv[:, 0:CPC])

        # ---------- main loop ----------
        with tc.tile_pool(name="rwork", bufs=2) as rwk, \
             tc.tile_pool(name="b0p", bufs=2) as b0p, \
             tc.tile_pool(name="qkp", bufs=2) as qkp, \
             tc.tile_pool(name="scp", bufs=6, space="PSUM") as scp, \
             tc.tile_pool(name="avp", bufs=2, space="PSUM") as avp, \
             tc.tile_pool(name="ptsp", bufs=4) as ptsp, \
             tc.tile_pool(name="stgp", bufs=2) as stgp, \
             tc.tile_pool(name="stats", bufs=4) as stp, \
             tc.tile_pool(name="nwork", bufs=2) as nwk:

            def pair_tiles(p):
                # per-pair q/k band layouts, auto-rotated (bufs=2):
                #  q_hl: [qhi|qlo]   (cheap-pass lhsT)
                #  q_hh: [qhi|qhi]   (S^T mm1 rhs)
                #  ql_b: [qlo; -mhat](S^T mm2 rhs, 65 partitions)
                #  k_hl: [khi|klo]   (S^T mm1 lhsT)
                #  k_hh: [khi|khi]   (cheap-pass rhs)
                #  k_ho: [khi; ones] (S^T mm2 lhsT, 65 partitions)
                t = {}
                for nm in ("q_hl", "q_hh", "k_hl", "k_hh"):
                    t[nm] = qkp.tile([128, 2, S], BF16, tag=nm, name=nm)
                for nm in ("ql_b", "k_ho"):
                    t[nm] = qkp.tile([65, 2, S], BF16, tag=nm, name=nm)
                return t

            def ones_memset(tl):
                nc.gpsimd.memset(tl["k_ho"][64:65, :, :], 1.0)

            def proj_gen(p, c, ws, tl, xc):
                """Q/K projection + rope for pair p, seq chunk c (xc
                preloaded). Yields between matmul bursts."""
                ssl = bass.ts(c, 512)
                for qk, (w_hi, w_lo) in enumerate(
                    ((ws["qh"], ws["ql"]), (ws["kh"], ws["kl"]))
                ):
                    pqk = scp.tile([128, 512], F32, tag="sc", name="pqk")
                    n = 0
                    for t in range(DSUB):
                        for lh, xi in ((w_hi, 0), (w_hi, 1), (w_lo, 0)):
                            nc.tensor.matmul(
                                pqk[:], lh[:, t, :], xc[:, xi, t, :],
                                start=(n == 0), stop=(n == 3 * DSUB - 1),
                            )
                            n += 1
                            if n % 6 == 0 and n < 18:
                                yield 6 * 512
                    yield 6 * 512
                    # rope (2 heads stacked on partitions). Compute engines
                    # can only shift partitions UPWARD (walrus crashes on
                    # out@lower<-in@higher), so the rotate-half splits:
                    # up-shifts on DVE (fast deps), down-shifts via 2 DMAs
                    # feeding same-row gpsimd muls. ACT keeps only the psum
                    # evacuation copy so queued attention exps behind it
                    # aren't head-of-line blocked on the rope chain.
                    f32c = rwk.tile([128, 512], F32, tag="f32c")
                    nc.scalar.copy(out=f32c[:], in_=pqk[:])
                    swp = rwk.tile([128, 512], F32, tag="swp")
                    sos = rwk.tile([128, 512], F32, tag="sos")
                    for a in (0, 2):  # up: even rows -> odd-row slots
                        # both inputs at base 32a (walrus requires equal
                        # input bases); only the output may shift up. The
                        # -sin here (vs +sin wanted) flips the sign of all
                        # odd-dim rows; cos_t is negated on those rows too,
                        # so q_odd/k_odd are both negated -> q.k unchanged.
                        nc.vector.tensor_mul(
                            swp[32 * a + 32:32 * a + 64, :],
                            f32c[32 * a:32 * a + 32, :],
                            sin_t[32 * a:32 * a + 32, ssl],
                        )
                    for a in (0, 2):  # down: odd rows via DMA, then mul
                        nc.sync.dma_start(
                            sos[32 * a:32 * a + 32, :],
                            f32c[32 * a + 32:32 * a + 64, :],
                        )
                        nc.gpsimd.tensor_mul(
                            swp[32 * a:32 * a + 32, :],
                            sos[32 * a:32 * a + 32, :],
                            sin_t[32 * a:32 * a + 32, ssl],
                        )
                    nc.gpsimd.tensor_mul(f32c[:], f32c[:], cos_t[:, ssl])
                    nc.gpsimd.tensor_add(swp[:], swp[:], f32c[:])
                    for sub in range(2):
                        hh = sub  # head slot within pair
                        band = swp[64 * sub:64 * sub + 64, :]
                        if sub == 0:
                            b0 = band
                        else:
                            b0t = b0p.tile([64, 512], F32, tag="b0t")
                            nc.sync.dma_start(b0t[:], band)
                            b0 = b0t[:]
                        if qk == 0:
                            q_hl, q_hh, ql_b = tl["q_hl"], tl["q_hh"], tl["ql_b"]
                            nc.scalar.copy(out=q_hl[0:64, hh, ssl], in_=b0)
                            nc.vector.tensor_tensor(
                                q_hl[64:128, hh, ssl], b0,
                                q_hl[0:64, hh, ssl], SUB,
                            )
                            nc.sync.dma_start(
                                q_hh[0:64, hh, ssl], q_hl[0:64, hh, ssl])
                            nc.sync.dma_start(
                                q_hh[64:128, hh, ssl], q_hl[0:64, hh, ssl])
                            nc.sync.dma_start(
                                ql_b[0:64, hh, ssl], q_hl[64:128, hh, ssl])
                        else:
                            k_hl, k_hh, k_ho = tl["k_hl"], tl["k_hh"], tl["k_ho"]
                            nc.scalar.copy(out=k_hl[0:64, hh, ssl], in_=b0)
                            nc.vector.tensor_tensor(
                                k_hl[64:128, hh, ssl], b0,
                                k_hl[0:64, hh, ssl], SUB,
                            )
                            nc.sync.dma_start(
                                k_hh[0:64, hh, ssl], k_hl[0:64, hh, ssl])
                            nc.sync.dma_start(
                                k_hh[64:128, hh, ssl], k_hl[0:64, hh, ssl])
                            nc.sync.dma_start(
                                k_ho[0:64, hh, ssl], k_hl[0:64, hh, ssl])

            def cheap_gen(hh, g, tl):
                """Cheap 2-term [q,k] pass for slot (head-in-pair hh, group g):
                row maxes -> -mhat bf16 row written to ql_b[64, hh, g*512:]."""
                q_hl, k_hh, ql_b = tl["q_hl"], tl["k_hh"], tl["ql_b"]
                stage = stgp.tile([128, 128], BF16, tag="stage")
                nc.gpsimd.memset(stage[:, 4:128], 0.0)
                for qtl in range(4):
                    qt = 4 * g + qtl
                    nk = (qt + 1) * 128
                    nblk = (nk + 511) // 512
                    qsl = bass.ts(qt, 128)
                    mxp = stp.tile([128, 4], F32, tag="mxp")
                    for b in range(nblk):
                        k0 = 512 * b
                        nn = min(512, nk - k0)
                        sc = scp.tile([128, 512], F32, tag="sc")
                        nc.tensor.matmul(
                            sc[:, 0:nn],
                            q_hl[:, hh, qsl], k_hh[:, hh, bass.ds(k0, nn)],
                            start=True, stop=True,
                        )
                        if b == nblk - 1:
                            # causal mask on the diagonal 128 cols
                            # (tensor_tensor_reduce would fuse this but
                            # crashes TRN2 hw)
                            nc.vector.tensor_tensor(
                                sc[:, bass.ds(nn - 128, 128)],
                                sc[:, bass.ds(nn - 128, 128)],
                                mask512[:, 384:512], ADD,
                            )
                        nc.vector.tensor_reduce(
                            mxp[:, b:b + 1], sc[:, 0:nn],
                            mybir.AxisListType.X, MAX,
                        )
                        yield nn
                    # combine partials, negate, round to bf16
                    nc.vector.tensor_reduce(
                        stage[:, qtl:qtl + 1], mxp[:, 0:nblk],
                        mybir.AxisListType.X, MAX, negate=True,
                    )
                # transpose [q,qtl] -> [qtl,q]; write -mhat row into ql_b
                outt = stgp.tile([128, 128], BF16, tag="outt")
                nc.sync.dma_start_transpose(outt[:], stage[:])
                nc.sync.dma_start(
                    ql_b[64:65, hh, bass.ts(g, 512)], outt[0:4, 0:128])

            def stav_gen(hh, g, tl, h_abs):
                """Precise S^T + exp + AV for slot (hh, g). Yields between
                matmul groups. st leads av by 2 k-tiles."""
                k_hl, k_ho = tl["k_hl"], tl["k_ho"]
                q_hh, ql_b = tl["q_hh"], tl["ql_b"]
                jmax = 4 * g + 3
                av = avp.tile([65, 512], F32, tag="av")
                gq0 = g * 512
                pend = deque()  # (j, pts_tile, q0, nq)

                def av_mm(j, pts_t, q0, nq):
                    nc.tensor.matmul(
                        av[:, bass.ds(q0, nq)],
                        v_ext[:, j, h_abs % NHC, :], pts_t[:, 0:nq],
                        start=(j == 0), stop=(j == jmax),
                        skip_group_check=True,
                    )

                for j in range(jmax + 1):
                    q0 = max(0, (j - 4 * g) * 128)
                    nq = 512 - q0
                    jsl = bass.ts(j, 128)
                    st = scp.tile([128, 512], F32, tag="sc")
                    nc.tensor.matmul(
                        st[:, bass.ds(q0, nq)],
                        k_hl[:, hh, jsl],
                        q_hh[:, hh, bass.ds(gq0 + q0, nq)],
                        start=True, stop=False,
                    )
                    nc.tensor.matmul(
                        st[:, bass.ds(q0, nq)],
                        k_ho[0:65, hh, jsl],
                        ql_b[0:65, hh, bass.ds(gq0 + q0, nq)],
                        start=False, stop=True,
                    )
                    if j >= 4 * g:  # diagonal tile: causal mask in [k,q]
                        nc.vector.tensor_tensor(
                            st[:, bass.ds(q0, 128)],
                            st[:, bass.ds(q0, 128)], mask_t[:], ADD,
                        )
                    pts_t = ptsp.tile([128, 512], BF16, tag="pts")
                    nc.scalar.activation(
                        pts_t[:, 0:nq], st[:, bass.ds(q0, nq)],
                        EXP, scale=0.125,
                    )
                    pend.append((j, pts_t, q0, nq))
                    yield 2 * nq
                    if len(pend) > 2:
                        av_mm(*pend.popleft())
                        yield 512
                while pend:
                    av_mm(*pend.popleft())
                    yield 512
                # normalize: row 64 holds the denominator
                dro = nwk.tile([1, 512], F32, tag="dro")
                nc.vector.tensor_copy(dro[:], av[64:65, :])
                rec = nwk.tile([1, 512], F32, tag="rec")
                nc.vector.reciprocal_approx_fast(out=rec[:], in_=dro[:])
                recb = nwk.tile([64, 512], F32, tag="recb")
                nc.gpsimd.partition_broadcast(recb[:], rec[0:1, :])
                hl, pr = h_abs % 2, h_abs // 2
                nc.vector.tensor_mul(
                    av_all[64 * hl:64 * hl + 64, pr, bass.ts(g, 512)],
                    av[0:64, :], recb[:],
                )

            # ---- weave scheduler ----
            # slots in order: for p, for g, for head-in-pair
            attnq = deque()   # active attention generators [(kind, gen)]
            state = {"attn_cols": 0, "proj_cols": 0}

            def pull(gen):
                try:
                    cols = next(gen[1])
                    state["attn_cols"] += cols
                    return True
                except StopIteration:
                    try:
                        attnq.remove(gen)
                    except ValueError:
                        pass
                    return False

            def pump_attn(target_ratio=1.9, max_units=10**9):
                """Advance attention gens: primary = head of queue, weave
                with the first independent 'cheap' gen behind it."""
                units = 0
                while attnq and units < max_units and (
                    state["attn_cols"] < target_ratio * state["proj_cols"]
                    or target_ratio < 0
                ):
                    primary = attnq[0]
                    if not pull(primary):
                        continue
                    units += 1
                    sec = None
                    for gq in list(attnq)[1:]:
                        if gq[0] == "cheap":
                            sec = gq
                            break
                    if sec is not None:
                        pull(sec)
                        units += 1

            ws_cur = ws_first
            tl_cur = pair_tiles(0)
            ones_memset(tl_cur)
            xc_cur = xc_first
            NCH = S // 512
            # stav generators wait one full chunk after their cheap pass so
            # the -mhat chain (DVE reduces -> XBAR -> row DMA, ~7us) is
            # always ready before the S^T mm2 reads it
            pendingA = deque()
            for p in range(NPAIR):
                ws_next = load_wslices(p + 1) if p < NPAIR - 1 else None
                tl_next = pair_tiles(p + 1) if p < NPAIR - 1 else None
                for c in range(NCH):
                    if c == NCH - 1 and tl_next is not None:
                        ones_memset(tl_next)
                    xc = xc_cur
                    if NCH * p + c < NPAIR * NCH - 1:
                        xc_cur = load_xc(c + 1)
                    for _cols in proj_gen(p, c, ws_cur, tl_cur, xc):
                        state["proj_cols"] += _cols
                        pump_attn(max_units=3)
                    g = c
                    for hh in range(2):
                        attnq.append(("cheap", cheap_gen(hh, g, tl_cur)))
                    while pendingA:
                        attnq.append(pendingA.popleft())
                    for hh in range(2):
                        pendingA.append(
                            ("stav", stav_gen(hh, g, tl_cur, 2 * p + hh)))
                    pump_attn()
                ws_cur, tl_cur = ws_next, tl_next
            # flush remaining attention
            while attnq:
                pump_attn(target_ratio=-1)
            attnq.extend(pendingA)
            pendingA.clear()
            while attnq:
                pump_attn(target_ratio=-1)

        # ---------- output projection ----------
        with tc.tile_pool(name="ops", bufs=2, space="PSUM") as ops, \
             tc.tile_pool(name="wop", bufs=1) as wop, \
             tc.tile_pool(name="owork", bufs=3) as owk:
            wo = load(wop, wo_d, "wo")
            for st in range(NQT):
                po = ops.tile([128, 2, 512], F32, tag="po")
                for half in range(2):
                    for p in range(NPAIR):
                        nc.tensor.matmul(
                            po[:, half, 0:384],
                            av_all[:, p, bass.ts(st, 128)],
                            wo[:, p, bass.ts(half, 384)],
                            start=(p == 0), stop=(p == NPAIR - 1),
                        )
                osb = owk.tile([128, D], F32, tag="osb")
                nc.scalar.copy(out=osb[:, 0:384], in_=po[:, 0, 0:384])
                nc.scalar.copy(out=osb[:, 384:768], in_=po[:, 1, 0:384])
                nc.sync.dma_start(out_d[bass.ts(st, 128), :], osb[:])

    nc.compile()
    return nc


def _rope_perm():
    p = np.zeros(DK, dtype=np.int64)
    for i in range(DK // 2):
        p[i] = 2 * i
        p[i + 32] = 2 * i + 1
    return p


def _split(a):
    hi = a.astype(bf16)
    lo = (a.astype(np.float32) - hi.astype(np.float32)).astype(bf16)
    return hi, lo


def _tile_din(a):
    # [768, F] -> [128, 6, F]
    return np.ascontiguousarray(a.reshape(DSUB, 128, -1).transpose(1, 0, 2))


def make_inputs(x, wq, wk, wv, wo, S):
    """Host-side prep: returns list of 8 in_maps (core = 2*b + g)."""
    perm = _rope_perm()
    pos = np.arange(S, dtype=np.float64)
    inv = 10000.0 ** (-2.0 * np.arange(DK // 2, dtype=np.float64) / DK)
    ang = pos[:, None] * inv[None, :]
    cosv = np.cos(ang).astype(np.float32).T  # [32, S]
    sinv = np.sin(ang).astype(np.float32).T
    cos_t = np.tile(
        np.concatenate([cosv, -cosv], axis=0), (2, 1)
    ).astype(np.float32)                                        # [128, S]
    sin_t = np.tile(
        np.concatenate([-sinv, sinv], axis=0), (2, 1)
    ).astype(np.float32)                                        # [128, S]
    # [zeros(384) | triu(-1e9, 1)]: right-aligned causal mask for the
    # cheap-max pass's last block ([q,k] orientation)
    mask512 = np.zeros((128, 512), np.float32)
    mask512[:, 384:] = np.triu(np.full((128, 128), -1e9, np.float32), 1)
    # [k,q] diagonal-tile mask: invalid k > q
    mask_t = np.tril(np.full((128, 128), -1e9, np.float32), -1)

    maps = []
    for b in range(B):
        xT = np.ascontiguousarray(x[b].T.astype(np.float32))  # [768, S]
        xh, xl = _split(xT)
        xh_t, xl_t = _tile_din(xh), _tile_din(xl)
        for g in range(2):
            hs = slice(g * CPC, (g + 1) * CPC)
            wqc = wq[hs].astype(np.float32).copy()
            wkc = wk[hs].astype(np.float32).copy()
            for arr in (wqc, wkc):
                for i in range(NHC):
                    blk = arr[i * DK:(i + 1) * DK].copy()
                    arr[i * DK:(i + 1) * DK] = blk[perm]
            wqh, wql = _split(wqc.T)  # [768, 384]
            wkh, wkl = _split(wkc.T)
            wvT = wv[hs].astype(np.float32).T.astype(bf16)
            woT = wo[:, hs].astype(np.float32).T.astype(bf16)  # [384, 768]
            maps.append({
                "xh": xh_t, "xl": xl_t,
                "wqh": _tile_din(wqh), "wql": _tile_din(wql),
                "wkh": _tile_din(wkh), "wkl": _tile_din(wkl),
                "wvT": _tile_din(wvT),
                "woT": np.ascontiguousarray(
                    woT.reshape(NPAIR, 128, D).transpose(1, 0, 2)),
                "cos_t": cos_t, "sin_t": sin_t,
                "mask512": mask512, "mask_t": mask_t,
            })
    return maps


_PROG = {}


def _prog(S):
    if S not in _PROG:
        _PROG[S] = _build(S)
    return _PROG[S]


def kernel(x, wq, wk, wv, wo, S=2048, trace=False):
    x = np.asarray(x, np.float32)
    nc = _prog(S)
    maps = make_inputs(x, np.asarray(wq), np.asarray(wk), np.asarray(wv),
                       np.asarray(wo), S)
    res = run_bass_kernel_spmd(nc, maps, list(range(8)), trace=trace)
    outs = []
    for b in range(B):
        outs.append(res.results[2 * b]["out"] + res.results[2 * b + 1]["out"])
    out = np.stack(outs)
    if trace:
        kernel.last_exec_time_ns = res.exec_time_ns
        kernel.last_results = res
    return out


# revision 32
# speedup vs baseline: 1.2487x; 1.2487x over previous
"""TRN2 Bass kernel for multi-head self-attention with RoPE (causal).

Problem: B=4, S=2048, D=768, H=12 heads of dk=64, fp32 in/out.

Sharding: 8 cores = 4 batches x 2 head-groups of 6 heads. Each core computes
QKV projections for its 6 heads, RoPE, causal attention, and a partial
output projection; the host sums the two partials per batch.

v4: TRANSPOSED-SCORES rewrite. The v3 kernel spent ~525us of Sync/Scalar
dispatch + 16 DMA queues on XBAR-transposing P [q,k] -> [k,q] for the AV
matmul, starving the PE (51% busy, p-state never ramped). v4 computes the
precise scores directly transposed, S^T[k,q] = K Q^T, so exp(S^T) IS the
AV rhs and no transpose of score-sized data ever happens:

  - cheap-max pass [q,k]: one 2-term matmul per 512-block
    ((qhi+qlo)@khi via band layout), DVE row-max (mask fused into the
    last block via tensor_tensor_reduce with a right-aligned [0|tri]
    mask), per-128-q-tile maxes -> negate -> bf16 -> one [128,128] XBAR
    transpose per (head, q-group) -> row [1,512] of -mhat.
    Empirically |rowmax_cheap - rowmax_precise| <= 7 raw-score units;
    the exp window tolerates ~60, and any consistent bias cancels in the
    normalization, so the cheap max only has to bound, not match.
  - precise S^T per k-tile j (same 3 terms as v3 = qhi@khi + qlo@khi +
    qhi@klo): mm1 = k_hl[khi|klo bands] x q_hh[qhi|qhi], mm2 =
    k_ho[khi;ones][65] x ql_b[qlo;-mhat][65] -- the 65th contract row
    adds -mhat to every score for free, solving the "per-column bias"
    problem of the transposed layout.
  - mask_t on diagonal tiles, ACT exp (scale=1/8, no bias) -> pts bf16,
    AV accumulates v_ext[128,65-with-ones] x pts into [65,512] psum;
    row 64 = softmax denominator for free; reciprocal+broadcast
    normalize into av_all.
  - PE p-state care: TRN2's tensor clock ramps 0.65->1.2->2.4GHz with
    ~3us of continuous busy; every idle gap halves throughput for the
    next 3us. A weave scheduler interleaves (proj bursts) x (cheap of
    slot t+1) x (S^T/AV of slot t) at matmul granularity so the PE
    stream stays dense and all DVE/ACT post-processing hides under it.
"""

import sys

sys.path.insert(0, "/opt/trn_rl_repo")

from collections import deque
from contextlib import ExitStack

import ml_dtypes
import numpy as np

import concourse.bass as bass
import concourse.tile as tile
from concourse import bacc, mybir
from concourse.bass_utils import run_bass_kernel_spmd

F32 = mybir.dt.float32
BF16 = mybir.dt.bfloat16
bf16 = ml_dtypes.bfloat16

B, D, H, DK = 4, 768, 12, 64
NHC = 6          # heads per core
NPAIR = 3        # head pairs per core
DSUB = 6         # d_in subtiles of 128
CPC = NHC * DK   # 384 head-dims per core

SUB = mybir.AluOpType.subtract
ADD = mybir.AluOpType.add
MAX = mybir.AluOpType.max
EXP = mybir.ActivationFunctionType.Exp


def _build(S=2048, trace_label=""):
    NQT = S // 128       # 16 q-tiles
    NG = NQT // 4        # 4 q-groups per head (512 q cols each)
    nc = bacc.Bacc("TRN2", target_bir_lowering=False, debug=False,
                   num_devices=8)

    def din(name, shape, dt):
        return nc.dram_tensor(name, shape, dt, kind="ExternalInput").ap()

    xh_d = din("xh", [128, DSUB, S], BF16)
    xl_d = din("xl", [128, DSUB, S], BF16)
    wqh_d = din("wqh", [128, DSUB, CPC], BF16)
    wql_d = din("wql", [128, DSUB, CPC], BF16)
    wkh_d = din("wkh", [128, DSUB, CPC], BF16)
    wkl_d = din("wkl", [128, DSUB, CPC], BF16)
    wv_d = din("wvT", [128, DSUB, CPC], BF16)
    wo_d = din("woT", [128, NPAIR, D], BF16)
    cos_d = din("cos_t", [128, S], F32)
    sin_d = din("sin_t", [128, S], F32)
    m512_d = din("mask512", [128, 512], F32)   # [zeros(384) | triu128]
    maskt_d = din("mask_t", [128, 128], F32)   # tril(-1e9, -1): [k,q] diag
    out_d = nc.dram_tensor("out", [S, D], F32, kind="ExternalOutput").ap()

    with tile.TileContext(nc) as tc, ExitStack() as ctx:
        # ---------- persistent SBUF ----------
        pers = ctx.enter_context(tc.tile_pool(name="pers", bufs=1))

        def load(pool, dr, name):
            t = pool.tile(list(dr.shape), dr.dtype, tag=f"L{name}")
            nc.sync.dma_start(t[:], dr[:])
            return t

        mask512 = load(pers, m512_d, "m512")
        mask_t = load(pers, maskt_d, "maskt")
        cos_t = load(pers, cos_d, "cos")
        sin_t = load(pers, sin_d, "sin")

        # v with an appended ones column per head: AV matmul row 64 = sum(p)
        v_ext = pers.tile([128, NQT, NHC, 65], BF16, tag="v_ext")
        av_all = pers.tile([128, NPAIR, S], BF16, tag="av_all")

        # pools that span the V phase and the main loop (prefetch of the
        # first Q/K weights + x chunk overlaps the V projection)
        wsl = ctx.enter_context(tc.tile_pool(name="wsl", bufs=2))
        bx = ctx.enter_context(tc.tile_pool(name="bx", bufs=2))

        def load_wslices(p):
            ws = {}
            for nm, dr in (("qh", wqh_d), ("ql", wql_d),
                           ("kh", wkh_d), ("kl", wkl_d)):
                t = wsl.tile([128, DSUB, 128], BF16, tag=f"w{nm}", name=nm)
                nc.sync.dma_start(t[:], dr[:, :, bass.ts(p, 128)])
                ws[nm] = t
            return ws

        def load_xc(c_seq):
            # prefetched one chunk ahead of its proj consumer
            xc = bx.tile([128, 2, DSUB, 512], BF16, tag="xc", name="xc")
            ssl = bass.ts(c_seq % (S // 512), 512)
            nc.sync.dma_start(xc[:, 0], xh_d[:, :, ssl])
            nc.sync.dma_start(xc[:, 1], xl_d[:, :, ssl])
            return xc

        ws_first = load_wslices(0)
        xc_first = load_xc(0)

        # ---------- V projection upfront ----------
        with tc.tile_pool(name="vw", bufs=1) as vw, \
             tc.tile_pool(name="vx", bufs=2) as vx, \
             tc.tile_pool(name="pv", bufs=2, space="PSUM") as pvs:
            wv = load(vw, wv_d, "wv")
            nc.gpsimd.memset(v_ext[:, :, :, 64:65], 1.0)

            def load_xv(sc_i):
                xv = vx.tile([128, DSUB, 512], BF16, tag="xv")
                nc.sync.dma_start(xv[:], xh_d[:, :, bass.ts(sc_i, 512)])
                return xv

            xv_cur = load_xv(0)
            for sc_i in range(S // 512):
                xv = xv_cur
                if sc_i < S // 512 - 1:
                    xv_cur = load_xv(sc_i + 1)
                for st4 in range(4):
                    st = sc_i * 4 + st4
                    psv = pvs.tile([128, 512], F32, tag="pv")
                    for t in range(DSUB):
                        nc.tensor.matmul(
                            psv[:, 0:CPC],
                            xv[:, t, bass.ts(st4, 128)], wv[:, t, :],
                            start=(t == 0), stop=(t == DSUB - 1),
                        )
                    nc.scalar.copy(out=v_ext[:, st, :, 0:64], in_=psv[:, 0:CPC])

        # ---------- main loop ----------
        with tc.tile_pool(name="rwork", bufs=2) as rwk, \
             tc.tile_pool(name="b0p", bufs=2) as b0p, \
             tc.tile_pool(name="qkp", bufs=2) as qkp, \
             tc.tile_pool(name="scp", bufs=4, space="PSUM") as scp, \
             tc.tile_pool(name="avp", bufs=2, space="PSUM") as avp, \
             tc.tile_pool(name="pqk", bufs=2, space="PSUM") as pps, \
             tc.tile_pool(name="ptsp", bufs=4) as ptsp, \
             tc.tile_pool(name="stgp", bufs=2) as stgp, \
             tc.tile_pool(name="stats", bufs=4) as stp, \
             tc.tile_pool(name="nwork", bufs=2) as nwk:

            def pair_tiles(p):
                # per-pair q/k band layouts, auto-rotated (bufs=2):
                #  q_hl: [qhi|qlo]   (cheap-pass lhsT)
                #  q_hh: [qhi|qhi]   (S^T mm1 rhs)
                #  ql_b: [qlo; -mhat](S^T mm2 rhs, 65 partitions)
                #  k_hl: [khi|klo]   (S^T mm1 lhsT)
                #  k_hh: [khi|khi]   (cheap-pass rhs)
                #  k_ho: [khi; ones] (S^T mm2 lhsT, 65 partitions)
                t = {}
                for nm in ("q_hl", "q_hh", "k_hl", "k_hh"):
                    t[nm] = qkp.tile([128, 2, S], BF16, tag=nm, name=nm)
                for nm in ("ql_b", "k_ho"):
                    t[nm] = qkp.tile([65, 2, S], BF16, tag=nm, name=nm)
                return t

            def ones_memset(tl):
                nc.gpsimd.memset(tl["k_ho"][64:65, :, :], 1.0)

            def proj_gen(p, c, ws, tl, xc):
                """Q/K projection + rope for pair p, seq chunk c (xc
                preloaded). Yields between matmul bursts. The writebacks
                are deferred past later yields so their ACT/DVE ops are
                emitted after the rope chain has mostly resolved -- an
                engine executes its queue in order, so emitting an op
                that will wait 4us blocks every op queued behind it."""
                ssl = bass.ts(c, 512)
                swps = []

                def rope_head(pqk):
                    # rope (2 heads stacked on partitions). Compute engines
                    # can only shift partitions UPWARD (walrus crashes on
                    # out@lower<-in@higher), so the rotate-half splits:
                    # up-shifts on DVE (fast deps), down-shifts via 2 DMAs
                    # feeding same-row gpsimd muls.
                    f32c = rwk.tile([128, 512], F32, tag="f32c", name="f32c")
                    nc.scalar.copy(out=f32c[:], in_=pqk[:])
                    swp = rwk.tile([128, 512], F32, tag="swp", name="swp")
                    sos = rwk.tile([128, 512], F32, tag="sos", name="sos")
                    for a in (0, 2):  # up: even rows -> odd-row slots
                        # both inputs at base 32a (walrus requires equal
                        # input bases); only the output may shift up. The
                        # -sin here (vs +sin wanted) flips the sign of all
                        # odd-dim rows; cos_t is negated on those rows too,
                        # so q_odd/k_odd are both negated -> q.k unchanged.
                        nc.vector.tensor_mul(
                            swp[32 * a + 32:32 * a + 64, :],
                            f32c[32 * a:32 * a + 32, :],
                            sin_t[32 * a:32 * a + 32, ssl],
                        )
                    for a in (0, 2):  # down: odd rows via DMA, then mul
                        nc.sync.dma_start(
                            sos[32 * a:32 * a + 32, :],
                            f32c[32 * a + 32:32 * a + 64, :],
                        )
                        nc.gpsimd.tensor_mul(
                            swp[32 * a:32 * a + 32, :],
                            sos[32 * a:32 * a + 32, :],
                            sin_t[32 * a:32 * a + 32, ssl],
                        )
                    nc.gpsimd.tensor_mul(f32c[:], f32c[:], cos_t[:, ssl])
                    nc.gpsimd.tensor_add(swp[:], swp[:], f32c[:])
                    return swp

                def writeback(qk, swp):
                    for sub in range(2):
                        hh = sub  # head slot within pair
                        band = swp[64 * sub:64 * sub + 64, :]
                        if sub == 0:
                            b0 = band
                        else:
                            b0t = b0p.tile([64, 512], F32, tag="b0t")
                            nc.sync.dma_start(b0t[:], band)
                            b0 = b0t[:]
                        if qk == 0:
                            q_hl, q_hh, ql_b = \
                                tl["q_hl"], tl["q_hh"], tl["ql_b"]
                            nc.scalar.copy(out=q_hl[0:64, hh, ssl], in_=b0)
                            nc.vector.tensor_tensor(
                                q_hl[64:128, hh, ssl], b0,
                                q_hl[0:64, hh, ssl], SUB,
                            )
                            nc.sync.dma_start(
                                q_hh[0:64, hh, ssl], q_hl[0:64, hh, ssl])
                            nc.sync.dma_start(
                                q_hh[64:128, hh, ssl], q_hl[0:64, hh, ssl])
                            nc.sync.dma_start(
                                ql_b[0:64, hh, ssl], q_hl[64:128, hh, ssl])
                        else:
                            k_hl, k_hh, k_ho = \
                                tl["k_hl"], tl["k_hh"], tl["k_ho"]
                            nc.scalar.copy(out=k_hl[0:64, hh, ssl], in_=b0)
                            nc.vector.tensor_tensor(
                                k_hl[64:128, hh, ssl], b0,
                                k_hl[0:64, hh, ssl], SUB,
                            )
                            nc.sync.dma_start(
                                k_hh[0:64, hh, ssl], k_hl[0:64, hh, ssl])
                            nc.sync.dma_start(
                                k_hh[64:128, hh, ssl], k_hl[0:64, hh, ssl])
                            nc.sync.dma_start(
                                k_ho[0:64, hh, ssl], k_hl[0:64, hh, ssl])

                for qk, (w_hi, w_lo) in enumerate(
                    ((ws["qh"], ws["ql"]), (ws["kh"], ws["kl"]))
                ):
                    pqk = pps.tile([128, 512], F32, tag="pp")
                    n = 0
                    for t in range(DSUB):
                        for lh, xi in ((w_hi, 0), (w_hi, 1), (w_lo, 0)):
                            nc.tensor.matmul(
                                pqk[:], lh[:, t, :], xc[:, xi, t, :],
                                start=(n == 0), stop=(n == 3 * DSUB - 1),
                            )
                            n += 1
                            if n % 6 == 0 and n < 18:
                                yield 6 * 512
                    swps.append(rope_head(pqk))
                    yield 6 * 512
                # q writeback: its rope chain had the whole k-proj to drain
                writeback(0, swps[0])
                yield 256
                # k writeback: one more pump window after its rope head
                writeback(1, swps[1])

            def cheap_gen(hh, g, tl):
                """Cheap 2-term [q,k] pass for slot (head-in-pair hh, group g):
                row maxes -> -mhat bf16 row written to ql_b[64, hh, g*512:]."""
                q_hl, k_hh, ql_b = tl["q_hl"], tl["k_hh"], tl["ql_b"]
                stage = stgp.tile([128, 128], BF16, tag="stage")
                nc.gpsimd.memset(stage[:, 4:128], 0.0)
                for qtl in range(4):
                    qt = 4 * g + qtl
                    nk = (qt + 1) * 128
                    nblk = (nk + 511) // 512
                    qsl = bass.ts(qt, 128)
                    mxp = stp.tile([128, 4], F32, tag="mxp")
                    for b in range(nblk):
                        k0 = 512 * b
                        nn = min(512, nk - k0)
                        sc = scp.tile([128, 512], F32, tag="sc")
                        nc.tensor.matmul(
                            sc[:, 0:nn],
                            q_hl[:, hh, qsl], k_hh[:, hh, bass.ds(k0, nn)],
                            start=True, stop=True,
                        )
                        if b == nblk - 1:
                            # causal mask on the diagonal 128 cols
                            # (tensor_tensor_reduce would fuse this but
                            # crashes TRN2 hw)
                            nc.vector.tensor_tensor(
                                sc[:, bass.ds(nn - 128, 128)],
                                sc[:, bass.ds(nn - 128, 128)],
                                mask512[:, 384:512], ADD,
                            )
                        nc.vector.tensor_reduce(
                            mxp[:, b:b + 1], sc[:, 0:nn],
                            mybir.AxisListType.X, MAX,
                        )
                        yield nn
                    # combine partials, negate, round to bf16
                    nc.vector.tensor_reduce(
                        stage[:, qtl:qtl + 1], mxp[:, 0:nblk],
                        mybir.AxisListType.X, MAX, negate=True,
                    )
                # transpose [q,qtl] -> [qtl,q]; write -mhat row into ql_b
                outt = stgp.tile([128, 128], BF16, tag="outt")
                nc.sync.dma_start_transpose(outt[:], stage[:])
                nc.sync.dma_start(
                    ql_b[64:65, hh, bass.ts(g, 512)], outt[0:4, 0:128])

            def stav_gen(hh, g, tl, h_abs):
                """Precise S^T + exp + AV for slot (hh, g). Yields between
                matmul groups. st leads av by 2 k-tiles."""
                k_hl, k_ho = tl["k_hl"], tl["k_ho"]
                q_hh, ql_b = tl["q_hh"], tl["ql_b"]
                jmax = 4 * g + 3
                av = avp.tile([65, 512], F32, tag="av")
                gq0 = g * 512
                pend = deque()  # (j, pts_tile, q0, nq)

                def av_mm(j, pts_t, q0, nq):
                    nc.tensor.matmul(
                        av[:, bass.ds(q0, nq)],
                        v_ext[:, j, h_abs % NHC, :], pts_t[:, 0:nq],
                        start=(j == 0), stop=(j == jmax),
                        skip_group_check=True,
                    )

                for j in range(jmax + 1):
                    q0 = max(0, (j - 4 * g) * 128)
                    nq = 512 - q0
                    jsl = bass.ts(j, 128)
                    st = scp.tile([128, 512], F32, tag="sc")
                    nc.tensor.matmul(
                        st[:, bass.ds(q0, nq)],
                        k_hl[:, hh, jsl],
                        q_hh[:, hh, bass.ds(gq0 + q0, nq)],
                        start=True, stop=False,
                    )
                    nc.tensor.matmul(
                        st[:, bass.ds(q0, nq)],
                        k_ho[0:65, hh, jsl],
                        ql_b[0:65, hh, bass.ds(gq0 + q0, nq)],
                        start=False, stop=True,
                    )
                    if j >= 4 * g:  # diagonal tile: causal mask in [k,q]
                        nc.vector.tensor_tensor(
                            st[:, bass.ds(q0, 128)],
                            st[:, bass.ds(q0, 128)], mask_t[:], ADD,
                        )
                    pts_t = ptsp.tile([128, 512], BF16, tag="pts")
                    nc.scalar.activation(
                        pts_t[:, 0:nq], st[:, bass.ds(q0, nq)],
                        EXP, scale=0.125,
                    )
                    pend.append((j, pts_t, q0, nq))
                    yield 2 * nq
                    if len(pend) > 2:
                        av_mm(*pend.popleft())
                        yield 512
                while pend:
                    av_mm(*pend.popleft())
                    yield 512
                # normalize: row 64 holds the denominator
                dro = nwk.tile([1, 512], F32, tag="dro")
                nc.vector.tensor_copy(dro[:], av[64:65, :])
                rec = nwk.tile([1, 512], F32, tag="rec")
                nc.vector.reciprocal_approx_fast(out=rec[:], in_=dro[:])
                recb = nwk.tile([64, 512], F32, tag="recb")
                nc.gpsimd.partition_broadcast(recb[:], rec[0:1, :])
                hl, pr = h_abs % 2, h_abs // 2
                nc.vector.tensor_mul(
                    av_all[64 * hl:64 * hl + 64, pr, bass.ts(g, 512)],
                    av[0:64, :], recb[:],
                )

            # ---- weave scheduler ----
            # slots in order: for p, for g, for head-in-pair
            attnq = deque()   # active attention generators [(kind, gen)]
            state = {"attn_cols": 0, "proj_cols": 0}

            def pull(gen):
                try:
                    cols = next(gen[1])
                    state["attn_cols"] += cols
                    return True
                except StopIteration:
                    try:
                        attnq.remove(gen)
                    except ValueError:
                        pass
                    return False

            def pump_attn(target_ratio=1.9, max_units=10**9):
                """Advance attention gens: primary = head of queue, weave
                with the first independent 'cheap' gen behind it."""
                units = 0
                while attnq and units < max_units and (
                    state["attn_cols"] < target_ratio * state["proj_cols"]
                    or target_ratio < 0
                ):
                    primary = attnq[0]
                    if not pull(primary):
                        continue
                    units += 1
                    sec = None
                    for gq in list(attnq)[1:]:
                        if gq[0] == "cheap":
                            sec = gq
                            break
                    if sec is not None:
                        pull(sec)
                        units += 1

            ws_cur = ws_first
            tl_cur = pair_tiles(0)
            ones_memset(tl_cur)
            xc_cur = xc_first
            NCH = S // 512
            # stav generators wait one full chunk after their cheap pass so
            # the -mhat chain (DVE reduces -> XBAR -> row DMA, ~7us) is
            # always ready before the S^T mm2 reads it
            pendingA = deque()
            for p in range(NPAIR):
                ws_next = load_wslices(p + 1) if p < NPAIR - 1 else None
                tl_next = pair_tiles(p + 1) if p < NPAIR - 1 else None
                for c in range(NCH):
                    if c == NCH - 1 and tl_next is not None:
                        ones_memset(tl_next)
                    xc = xc_cur
                    if NCH * p + c < NPAIR * NCH - 1:
                        xc_cur = load_xc(c + 1)
                    for _cols in proj_gen(p, c, ws_cur, tl_cur, xc):
                        state["proj_cols"] += _cols
                        pump_attn(max_units=3)
                    g = c
                    for hh in range(2):
                        attnq.append(("cheap", cheap_gen(hh, g, tl_cur)))
                    while pendingA:
                        attnq.append(pendingA.popleft())
                    for hh in range(2):
                        pendingA.append(
                            ("stav", stav_gen(hh, g, tl_cur, 2 * p + hh)))
                    pump_attn()
                ws_cur, tl_cur = ws_next, tl_next
            # flush remaining attention
            while attnq:
                pump_attn(target_ratio=-1)
            attnq.extend(pendingA)
            pendingA.clear()
            while attnq:
                pump_attn(target_ratio=-1)

        # ---------- output projection ----------
        with tc.tile_pool(name="ops", bufs=2, space="PSUM") as ops, \
             tc.tile_pool(name="wop", bufs=1) as wop, \
             tc.tile_pool(name="owork", bufs=3) as owk:
            wo = load(wop, wo_d, "wo")
            for st in range(NQT):
                po = ops.tile([128, 2, 512], F32, tag="po")
                for half in range(2):
                    for p in range(NPAIR):
                        nc.tensor.matmul(
                            po[:, half, 0:384],
                            av_all[:, p, bass.ts(st, 128)],
                            wo[:, p, bass.ts(half, 384)],
                            start=(p == 0), stop=(p == NPAIR - 1),
                        )
                osb = owk.tile([128, D], F32, tag="osb")
                nc.scalar.copy(out=osb[:, 0:384], in_=po[:, 0, 0:384])
                nc.scalar.copy(out=osb[:, 384:768], in_=po[:, 1, 0:384])
                nc.sync.dma_start(out_d[bass.ts(st, 128), :], osb[:])

    nc.compile()
    return nc


def _rope_perm():
    p = np.zeros(DK, dtype=np.int64)
    for i in range(DK // 2):
        p[i] = 2 * i
        p[i + 32] = 2 * i + 1
    return p


def _split(a):
    hi = a.astype(bf16)
    lo = (a.astype(np.float32) - hi.astype(np.float32)).astype(bf16)
    return hi, lo


def _tile_din(a):
    # [768, F] -> [128, 6, F]
    return np.ascontiguousarray(a.reshape(DSUB, 128, -1).transpose(1, 0, 2))


def make_inputs(x, wq, wk, wv, wo, S):
    """Host-side prep: returns list of 8 in_maps (core = 2*b + g)."""
    perm = _rope_perm()
    pos = np.arange(S, dtype=np.float64)
    inv = 10000.0 ** (-2.0 * np.arange(DK // 2, dtype=np.float64) / DK)
    ang = pos[:, None] * inv[None, :]
    cosv = np.cos(ang).astype(np.float32).T  # [32, S]
    sinv = np.sin(ang).astype(np.float32).T
    cos_t = np.tile(
        np.concatenate([cosv, -cosv], axis=0), (2, 1)
    ).astype(np.float32)                                        # [128, S]
    sin_t = np.tile(
        np.concatenate([-sinv, sinv], axis=0), (2, 1)
    ).astype(np.float32)                                        # [128, S]
    # [zeros(384) | triu(-1e9, 1)]: right-aligned causal mask for the
    # cheap-max pass's last block ([q,k] orientation)
    mask512 = np.zeros((128, 512), np.float32)
    mask512[:, 384:] = np.triu(np.full((128, 128), -1e9, np.float32), 1)
    # [k,q] diagonal-tile mask: invalid k > q
    mask_t = np.tril(np.full((128, 128), -1e9, np.float32), -1)

    maps = []
    for b in range(B):
        xT = np.ascontiguousarray(x[b].T.astype(np.float32))  # [768, S]
        xh, xl = _split(xT)
        xh_t, xl_t = _tile_din(xh), _tile_din(xl)
        for g in range(2):
            hs = slice(g * CPC, (g + 1) * CPC)
            wqc = wq[hs].astype(np.float32).copy()
            wkc = wk[hs].astype(np.float32).copy()
            for arr in (wqc, wkc):
                for i in range(NHC):
                    blk = arr[i * DK:(i + 1) * DK].copy()
                    arr[i * DK:(i + 1) * DK] = blk[perm]
            wqh, wql = _split(wqc.T)  # [768, 384]
            wkh, wkl = _split(wkc.T)
            wvT = wv[hs].astype(np.float32).T.astype(bf16)
            woT = wo[:, hs].astype(np.float32).T.astype(bf16)  # [384, 768]
            maps.append({
                "xh": xh_t, "xl": xl_t,
                "wqh": _tile_din(wqh), "wql": _tile_din(wql),
                "wkh": _tile_din(wkh), "wkl": _tile_din(wkl),
                "wvT": _tile_din(wvT),
                "woT": np.ascontiguousarray(
                    woT.reshape(NPAIR, 128, D).transpose(1, 0, 2)),
                "cos_t": cos_t, "sin_t": sin_t,
                "mask512": mask512, "mask_t": mask_t,
            })
    return maps


_PROG = {}


def _prog(S):
    if S not in _PROG:
        _PROG[S] = _build(S)
    return _PROG[S]


def kernel(x, wq, wk, wv, wo, S=2048, trace=False):
    x = np.asarray(x, np.float32)
    nc = _prog(S)
    maps = make_inputs(x, np.asarray(wq), np.asarray(wk), np.asarray(wv),
                       np.asarray(wo), S)
    res = run_bass_kernel_spmd(nc, maps, list(range(8)), trace=trace)
    outs = []
    for b in range(B):
        outs.append(res.results[2 * b]["out"] + res.results[2 * b + 1]["out"])
    out = np.stack(outs)
    if trace:
        kernel.last_exec_time_ns = res.exec_time_ns
        kernel.last_results = res
    return out
